# revision 26
# baseline (speedup 1.0000x reference)
"""ObjectAttentionBlock2D TRN2 kernel.

Reference computation (per batch b):
    xf    = x[b].reshape(C, N)                  # C=512, N=128*128=16384
    pf    = proxy[b,:,:,0]                      # [C, K], K=64
    query = Wq @ xf + bq                        # [Ck=256, N]
    keym  = Wk @ pf + bk                        # [Ck, K]
    value = (Wv @ pf + bv).T                    # [K, Cv=256]
    sim   = softmax_k(query.T @ keym / 16)      # [N, K]
    ctx   = sim @ value                         # [N, Cv]
    out   = Wo @ ctx.T + bo                     # [C, N]

Sharding: data-parallel over batch. B=8 batches -> 8 NeuronCores, one image
per core, no collectives. Weights are replicated (host pre-transposes them so
the contraction dim is the SBUF partition dim).

Per-core pipeline over 64 pixel tiles of F=256 columns:
  Q proj (8 fp16 MMs, contract C=512) -> +bq fused in ACT PSUM->SBUF copy
  simT [K=64, F] (2 f32r MMs, contract Ck) -> ACT exp(x/16) -> f32r SBUF
  denom = ones64^T @ expP (1 MM) -> DVE reciprocal -> K=1 broadcast MM
  expPn = expP * recip (DVE)
  ctxT [Cv, F] (2 f32r MMs, contract K=64) -> ACT copy to SBUF
  out [C, F] (8 f32r MMs, contract Cv) -> +bo fused in DVE copy -> DMA out
keym/value are precomputed once per core with biases folded in via K=1
matmul accumulation (bias outer-product with a ones row).

Precision: x/Wq/Wk/Wv/pf are cast to fp16 on the host (halves the dominant
x DMA stream; fp16's 10-bit mantissa covers this data's range); everything
downstream runs float32r (1 cycle/row at N>=256 vs 4 for plain fp32) with
fp32 PSUM accumulation. End-to-end max rel err vs the fp32 reference ~6e-4.

DMA layout: x-in on gpsimd/SWDGE (latency-tolerant prefetch), out on the
dedicated SP HWDGE queue (keeps out dispatch off the critical path), setup
constants packed into 3 DMAs (each HWDGE dispatch costs ~625ns serialized).
Cost-model (TimelineSim) exec: ~172 us/core; PE busy ~153 us (89%).
"""

import numpy as np

import concourse.bacc as bacc
import concourse.mybir as mybir
import concourse.tile as tile
from concourse import bass_utils

F32 = mybir.dt.float32
F32R = mybir.dt.float32r
F16 = mybir.dt.float16

B, C, H, W = 8, 512, 128, 128
N = H * W                    # 16384 pixels per image
CK, CV, K = 256, 256, 64
P = 128                      # SBUF partitions
F = 256                      # pixel-tile width
NT = N // F                  # 64 tiles
CI_CH = C // P               # 4 contraction chunks over C
Q_CH = CK // P               # 2 chunks over Ck
V_CH = CV // P               # 2 chunks over Cv
O_CH = C // P                # 4 chunks over output C
SCALE = CK ** -0.5           # 1/16

_CACHED = None


def _build():
    nc = bacc.Bacc("TRN2", target_bir_lowering=False, debug=False)

    X = nc.dram_tensor("x", [C, N], F16, kind="ExternalInput").ap()
    PF = nc.dram_tensor("pf", [C, K], F32, kind="ExternalInput").ap()
    WQT = nc.dram_tensor("wqT", [C, CK], F16, kind="ExternalInput").ap()
    WKT = nc.dram_tensor("wkT", [C, CK], F32, kind="ExternalInput").ap()
    WVT = nc.dram_tensor("wvT", [C, CV], F32, kind="ExternalInput").ap()
    WOT = nc.dram_tensor("woT", [CV, C], F32, kind="ExternalInput").ap()
    BQ = nc.dram_tensor("bq", [CK], F32, kind="ExternalInput").ap()
    BK = nc.dram_tensor("bk", [1, CK], F32, kind="ExternalInput").ap()
    BV = nc.dram_tensor("bv", [1, CV], F32, kind="ExternalInput").ap()
    BO = nc.dram_tensor("bo", [C], F32, kind="ExternalInput").ap()
    ONESR = nc.dram_tensor("ones_row", [1, 256], F32, kind="ExternalInput").ap()
    ONESC = nc.dram_tensor("ones_col", [K, 1], F32, kind="ExternalInput").ap()
    OUT = nc.dram_tensor("out", [C, N], F32, kind="ExternalOutput").ap()

    x_r = X.rearrange("(co p) n -> p co n", p=P)       # [128, 4, N]
    out_r = OUT.rearrange("(oo p) n -> p oo n", p=P)                 # [128, 4, N]

    with tile.TileContext(nc) as tc:
        with tc.tile_pool(name="const", bufs=1) as cp:
            wq = cp.tile([P, CI_CH, CK], F16)
            nc.sync.dma_start(wq, WQT.rearrange("(co p) q -> p co q", p=P))
            wk = cp.tile([P, CI_CH, CK], F32R)
            nc.scalar.dma_start(wk, WKT.bitcast(F32R).rearrange("(co p) q -> p co q", p=P))
            wv = cp.tile([P, CI_CH, CV], F32R)
            nc.scalar.dma_start(wv, WVT.bitcast(F32R).rearrange("(co p) v -> p co v", p=P))
            wo = cp.tile([P, V_CH, C], F32R)
            nc.scalar.dma_start(wo, WOT.bitcast(F32R).rearrange("(vo p) o -> p vo o", p=P))
            pf = cp.tile([P, CI_CH, K], F32R)
            nc.scalar.dma_start(pf, PF.bitcast(F32R).rearrange("(co p) k -> p co k", p=P))
            bq = cp.tile([P, Q_CH], F32)
            nc.scalar.dma_start(bq, BQ.rearrange("(qo p) -> p qo", p=P))
            bo = cp.tile([P, O_CH], F32)
            nc.scalar.dma_start(bo, BO.rearrange("(oo p) -> p oo", p=P))
            bk_row = cp.tile([1, CK], F32R)
            nc.scalar.dma_start(bk_row, BK.bitcast(F32R))
            bv_row = cp.tile([1, CV], F32R)
            nc.scalar.dma_start(bv_row, BV.bitcast(F32R))
            ones_row = cp.tile([1, 256], F32R)
            nc.scalar.dma_start(ones_row, ONESR.bitcast(F32R))
            ones_col = cp.tile([K, 1], F32R)
            nc.scalar.dma_start(ones_col, ONESC.bitcast(F32R))

            keym = cp.tile([P, Q_CH, K], F32R)    # [q-part, q-chunk, k]
            value = cp.tile([K, CV], F32R)        # [k, v]

            # ---- one-time: keym = Wk @ pf + bk, value[k,v] = (Wv @ pf + bv)[v,k]
            with tc.tile_pool(name="setup_ps", bufs=1, space="PSUM") as sps:
                kps = sps.tile([P, Q_CH, K], F32)
                for qi in range(Q_CH):
                    for ci in range(CI_CH):
                        nc.tensor.matmul(
                            kps[:, qi, :],
                            wk[:, ci, qi * P:(qi + 1) * P],
                            pf[:, ci, :],
                            start=(ci == 0), stop=False,
                        )
                    # += bk[q] * ones[k]
                    nc.tensor.matmul(
                        kps[:, qi, :],
                        bk_row[:, qi * P:(qi + 1) * P],
                        ones_row[:, :K],
                        start=False, stop=True,
                    )
                nc.vector.tensor_copy(keym, kps)

                vps = sps.tile([K, CV], F32)
                for ci in range(CI_CH):
                    nc.tensor.matmul(
                        vps, pf[:, ci, :], wv[:, ci, :],
                        start=(ci == 0), stop=False,
                    )
                # += ones[k] * bv[v]
                nc.tensor.matmul(
                    vps, ones_row[:, :K], bv_row, start=False, stop=True
                )
                nc.vector.tensor_copy(value, vps)

            # ---- steady-state pipeline over pixel tiles
            with (
                tc.tile_pool(name="xin", bufs=9) as xp,
                tc.tile_pool(name="qsb", bufs=3) as qp,
                tc.tile_pool(name="esb", bufs=3) as ep,
                tc.tile_pool(name="rsb", bufs=3) as rp,
                tc.tile_pool(name="ensb", bufs=3) as enp,
                tc.tile_pool(name="ctxsb", bufs=3) as ctxp,
                tc.tile_pool(name="outsb", bufs=3) as outp,
                tc.tile_pool(name="qps", bufs=1, space="PSUM") as qps,
                tc.tile_pool(name="sdps", bufs=1, space="PSUM") as sdps,
                tc.tile_pool(name="denps", bufs=1, space="PSUM") as denps,
                tc.tile_pool(name="denps", bufs=1, space="PSUM") as denps,
                tc.tile_pool(name="rbps", bufs=1, space="PSUM") as rbps,
                tc.tile_pool(name="ctxps", bufs=2, space="PSUM") as ctxps,
                tc.tile_pool(name="outps", bufs=1, space="PSUM") as outps,
            ):
                for t in range(NT):
                    n0 = t * F

                    x_t = xp.tile([P, CI_CH, F], F16, tag="x")
                    nc.gpsimd.dma_start(x_t, x_r[:, :, n0:n0 + F])

                    # Q = Wq @ x  -> [256, F] as 2 chunks
                    q_ps = qps.tile([P, Q_CH, F], F32, tag="qps")
                    for qi in range(Q_CH):
                        for ci in range(CI_CH):
                            nc.tensor.matmul(
                                q_ps[:, qi, :],
                                wq[:, ci, qi * P:(qi + 1) * P],
                                x_t[:, ci, :],
                                start=(ci == 0), stop=(ci == CI_CH - 1),
                            )
                    q_sb = qp.tile([P, Q_CH, F], F32R, tag="q")
                    for qi in range(Q_CH):
                        nc.scalar.activation(
                            q_sb[:, qi, :], q_ps[:, qi, :],
                            mybir.ActivationFunctionType.Identity,
                            bias=bq[:, qi:qi + 1],
                        )

                    # simT[k, n] = keym^T-contract-q @ Q
                    sim = sdps.tile([K, F], F32, tag="sd")
                    den = denps.tile([1, F], F32, tag="den")
                    for qi in range(Q_CH):
                        nc.tensor.matmul(
                            sim, keym[:, qi, :], q_sb[:, qi, :],
                            start=(qi == 0), stop=(qi == Q_CH - 1),
                        )
                    e = ep.tile([K, F], F32R, tag="e")
                    nc.scalar.activation(
                        e, sim, mybir.ActivationFunctionType.Exp, scale=SCALE
                    )
                    nc.tensor.matmul(den, ones_col, e, start=True, stop=True)
                    r_sb = rp.tile([1, F], F32R, tag="r")
                    with nc.allow_low_precision(reason="f32r is 4-byte fp32"):
                        nc.vector.reciprocal(r_sb, den)
                    rb_ps = rbps.tile([K, F], F32, tag="rb")
                    nc.tensor.matmul(rb_ps, ones_row[:, :K], r_sb, start=True, stop=True)
                    en = enp.tile([K, F], F32R, tag="en")
                    nc.vector.tensor_tensor(en, rb_ps, e, mybir.AluOpType.mult)

                    # ctxT[v, n] = value^T-contract-k @ expPn -> [256, F]
                    ctx_ps = ctxps.tile([P, V_CH, F], F32, tag="ctxps")
                    for vi in range(V_CH):
                        nc.tensor.matmul(
                            ctx_ps[:, vi, :],
                            value[:, vi * P:(vi + 1) * P],
                            en,
                            start=True, stop=True,
                        )
                    ctx_sb = ctxp.tile([P, V_CH, F], F32R, tag="ctx")
                    nc.scalar.copy(ctx_sb, ctx_ps)

                    # out = Wo @ ctx -> [512, F]
                    out_ps = outps.tile([P, O_CH, F], F32, tag="outps")
                    for oi in range(O_CH):
                        for vi in range(V_CH):
                            nc.tensor.matmul(
                                out_ps[:, oi, :],
                                wo[:, vi, oi * P:(oi + 1) * P],
                                ctx_sb[:, vi, :],
                                start=(vi == 0), stop=(vi == V_CH - 1),
                            )
                    out_sb = outp.tile([P, O_CH, F], F32, tag="out")
                    nc.vector.tensor_tensor(
                        out_sb, out_ps,
                        bo[:, :, None].to_broadcast([P, O_CH, F]),
                        mybir.AluOpType.add,
                    )
                    nc.sync.dma_start(out_r[:, :, n0:n0 + F], out_sb)

    nc.compile()
    return nc


def _get_nc():
    global _CACHED
    if _CACHED is None:
        _CACHED = _build()
    return _CACHED


def kernel(x, proxy, Wq, bq, Wk, bk, Wv, bv, Wo, bo, **run_kwargs):
    nc = _get_nc()

    shared = {
        "wqT": np.ascontiguousarray(Wq.T).astype(np.float16),
        "wkT": np.ascontiguousarray(Wk.T).astype(np.float32),
        "wvT": np.ascontiguousarray(Wv.T).astype(np.float32),
        "woT": np.ascontiguousarray(Wo.T).astype(np.float32),
        "bq": np.ascontiguousarray(bq).astype(np.float32),
        "bk": np.ascontiguousarray(bk).astype(np.float32).reshape(1, CK),
        "bv": np.ascontiguousarray(bv).astype(np.float32).reshape(1, CV),
        "bo": np.ascontiguousarray(bo).astype(np.float32),
        "ones_row": np.ones((1, 256), np.float32),
        "ones_col": np.ones((K, 1), np.float32),
    }
    in_maps = []
    for b in range(B):
        m = dict(shared)
        m["x"] = np.ascontiguousarray(x[b]).reshape(C, N).astype(np.float16)
        m["pf"] = np.ascontiguousarray(proxy[b, :, :, 0]).astype(np.float32)
        in_maps.append(m)

    res = bass_utils.run_bass_kernel_spmd(
        nc, in_maps, core_ids=list(range(B)), **run_kwargs
    )
    out = np.stack([res.results[b]["out"] for b in range(B)], axis=0)
    if run_kwargs:
        kernel.last_results = res
    return out.reshape(B, C, H, W)


# revision 27
# speedup vs baseline: 1.3392x; 1.3392x over previous
"""ObjectAttentionBlock2D TRN2 kernel.

Reference computation (per batch b):
    xf    = x[b].reshape(C, N)                  # C=512, N=128*128=16384
    pf    = proxy[b,:,:,0]                      # [C, K], K=64
    query = Wq @ xf + bq                        # [Ck=256, N]
    keym  = Wk @ pf + bk                        # [Ck, K]
    value = (Wv @ pf + bv).T                    # [K, Cv=256]
    sim   = softmax_k(query.T @ keym / 16)      # [N, K]
    ctx   = sim @ value                         # [N, Cv]
    out   = Wo @ ctx.T + bo                     # [C, N]

Sharding: data-parallel over batch. B=8 batches -> 8 NeuronCores, one image
per core, no collectives. Weights are replicated (host pre-transposes them so
the contraction dim is the SBUF partition dim).

Per-core pipeline over 64 pixel tiles of F=256 columns:
  Q proj (8 fp16 MMs, contract C=512) -> +bq fused in ACT PSUM->SBUF copy
  simT [K=64, F] (2 f32r MMs, contract Ck) -> ACT exp(x/16) -> f32r SBUF
  denom = ones64^T @ expP (1 MM) -> DVE reciprocal -> K=1 broadcast MM
  expPn = expP * recip (DVE)
  ctxT [Cv, F] (2 f32r MMs, contract K=64) -> ACT copy to SBUF
  out [C, F] (8 f32r MMs, contract Cv) -> +bo fused in DVE copy -> DMA out
keym/value are precomputed once per core with biases folded in via K=1
matmul accumulation (bias outer-product with a ones row).

Precision: x/Wq/Wk/Wv/pf are cast to fp16 on the host (halves the dominant
x DMA stream; fp16's 10-bit mantissa covers this data's range); everything
downstream runs float32r (1 cycle/row at N>=256 vs 4 for plain fp32) with
fp32 PSUM accumulation. End-to-end max rel err vs the fp32 reference ~6e-4.

DMA layout: x-in on gpsimd/SWDGE (latency-tolerant prefetch), out on the
dedicated SP HWDGE queue (keeps out dispatch off the critical path), setup
constants packed into 3 DMAs (each HWDGE dispatch costs ~625ns serialized).
Cost-model (TimelineSim) exec: ~172 us/core; PE busy ~153 us (89%).
"""

import numpy as np

import concourse.bacc as bacc
import concourse.mybir as mybir
import concourse.tile as tile
from concourse import bass_utils

F32 = mybir.dt.float32
F32R = mybir.dt.float32r
F16 = mybir.dt.float16

B, C, H, W = 8, 512, 128, 128
N = H * W                    # 16384 pixels per image
CK, CV, K = 256, 256, 64
P = 128                      # SBUF partitions
F = 256                      # pixel-tile width
NT = N // F                  # 64 tiles
CI_CH = C // P               # 4 contraction chunks over C
Q_CH = CK // P               # 2 chunks over Ck
V_CH = CV // P               # 2 chunks over Cv
O_CH = C // P                # 4 chunks over output C
SCALE = CK ** -0.5           # 1/16

_CACHED = None


def _build():
    nc = bacc.Bacc("TRN2", target_bir_lowering=False, debug=False)

    X = nc.dram_tensor("x", [C, N], F16, kind="ExternalInput").ap()
    PF = nc.dram_tensor("pf", [C, K], F32, kind="ExternalInput").ap()
    WQT = nc.dram_tensor("wqT", [C, CK], F16, kind="ExternalInput").ap()
    WKT = nc.dram_tensor("wkT", [C, CK], F32, kind="ExternalInput").ap()
    WVT = nc.dram_tensor("wvT", [C, CV], F32, kind="ExternalInput").ap()
    WOT = nc.dram_tensor("woT", [CV, C], F32, kind="ExternalInput").ap()
    BQ = nc.dram_tensor("bq", [CK], F32, kind="ExternalInput").ap()
    BK = nc.dram_tensor("bk", [1, CK], F32, kind="ExternalInput").ap()
    BV = nc.dram_tensor("bv", [1, CV], F32, kind="ExternalInput").ap()
    BO = nc.dram_tensor("bo", [C], F32, kind="ExternalInput").ap()
    ONESR = nc.dram_tensor("ones_row", [1, 256], F32, kind="ExternalInput").ap()
    ONESC = nc.dram_tensor("ones_col", [K, 1], F32, kind="ExternalInput").ap()
    OUT = nc.dram_tensor("out", [C, N], F32, kind="ExternalOutput").ap()

    x_r = X.rearrange("(co p) n -> p co n", p=P)       # [128, 4, N]
    out_r = OUT.rearrange("(oo p) n -> p oo n", p=P)                 # [128, 4, N]

    with tile.TileContext(nc) as tc:
        with tc.tile_pool(name="const", bufs=1) as cp:
            wq = cp.tile([P, CI_CH, CK], F16)
            nc.sync.dma_start(wq, WQT.rearrange("(co p) q -> p co q", p=P))
            wk = cp.tile([P, CI_CH, CK], F32R)
            nc.scalar.dma_start(wk, WKT.bitcast(F32R).rearrange("(co p) q -> p co q", p=P))
            wv = cp.tile([P, CI_CH, CV], F32R)
            nc.scalar.dma_start(wv, WVT.bitcast(F32R).rearrange("(co p) v -> p co v", p=P))
            wo = cp.tile([P, V_CH, C], F32R)
            nc.scalar.dma_start(wo, WOT.bitcast(F32R).rearrange("(vo p) o -> p vo o", p=P))
            pf = cp.tile([P, CI_CH, K], F32R)
            nc.scalar.dma_start(pf, PF.bitcast(F32R).rearrange("(co p) k -> p co k", p=P))
            bq = cp.tile([P, Q_CH], F32)
            nc.scalar.dma_start(bq, BQ.rearrange("(qo p) -> p qo", p=P))
            bo = cp.tile([P, O_CH], F32)
            nc.scalar.dma_start(bo, BO.rearrange("(oo p) -> p oo", p=P))
            bk_row = cp.tile([1, CK], F32R)
            nc.scalar.dma_start(bk_row, BK.bitcast(F32R))
            bv_row = cp.tile([1, CV], F32R)
            nc.scalar.dma_start(bv_row, BV.bitcast(F32R))
            ones_row = cp.tile([1, 256], F32R)
            nc.scalar.dma_start(ones_row, ONESR.bitcast(F32R))
            ones_col = cp.tile([K, 1], F32R)
            nc.scalar.dma_start(ones_col, ONESC.bitcast(F32R))

            keym = cp.tile([P, Q_CH, K], F32R)    # [q-part, q-chunk, k]
            value = cp.tile([K, CV], F32R)        # [k, v]

            # ---- one-time: keym = Wk @ pf + bk, value[k,v] = (Wv @ pf + bv)[v,k]
            with tc.tile_pool(name="setup_ps", bufs=1, space="PSUM") as sps:
                kps = sps.tile([P, Q_CH, K], F32)
                for qi in range(Q_CH):
                    for ci in range(CI_CH):
                        nc.tensor.matmul(
                            kps[:, qi, :],
                            wk[:, ci, qi * P:(qi + 1) * P],
                            pf[:, ci, :],
                            start=(ci == 0), stop=False,
                        )
                    # += bk[q] * ones[k]
                    nc.tensor.matmul(
                        kps[:, qi, :],
                        bk_row[:, qi * P:(qi + 1) * P],
                        ones_row[:, :K],
                        start=False, stop=True,
                    )
                nc.vector.tensor_copy(keym, kps)

                vps = sps.tile([K, CV], F32)
                for ci in range(CI_CH):
                    nc.tensor.matmul(
                        vps, pf[:, ci, :], wv[:, ci, :],
                        start=(ci == 0), stop=False,
                    )
                # += ones[k] * bv[v]
                nc.tensor.matmul(
                    vps, ones_row[:, :K], bv_row, start=False, stop=True
                )
                nc.vector.tensor_copy(value, vps)

            # ---- steady-state pipeline over pixel tiles
            with (
                tc.tile_pool(name="xin", bufs=9) as xp,
                tc.tile_pool(name="qsb", bufs=4) as qp,
                tc.tile_pool(name="esb", bufs=4) as ep,
                tc.tile_pool(name="rsb", bufs=4) as rp,
                tc.tile_pool(name="ensb", bufs=4) as enp,
                tc.tile_pool(name="ctxsb", bufs=4) as ctxp,
                tc.tile_pool(name="outsb", bufs=5) as outp,
                tc.tile_pool(name="qps", bufs=2, space="PSUM") as qps,
                tc.tile_pool(name="sdps", bufs=1, space="PSUM") as sdps,
                tc.tile_pool(name="denps", bufs=1, space="PSUM") as denps,
                tc.tile_pool(name="denps", bufs=1, space="PSUM") as denps,
                tc.tile_pool(name="rbps", bufs=1, space="PSUM") as rbps,
                tc.tile_pool(name="ctxps", bufs=1, space="PSUM") as ctxps,
                tc.tile_pool(name="outps", bufs=1, space="PSUM") as outps,
            ):
                for t in range(NT):
                    n0 = t * F

                    x_t = xp.tile([P, CI_CH, F], F16, tag="x")
                    nc.gpsimd.dma_start(x_t, x_r[:, :, n0:n0 + F])

                    # Q = Wq @ x  -> [256, F] as 2 chunks
                    q_ps = qps.tile([P, Q_CH, F], F32, tag="qps")
                    for qi in range(Q_CH):
                        for ci in range(CI_CH):
                            nc.tensor.matmul(
                                q_ps[:, qi, :],
                                wq[:, ci, qi * P:(qi + 1) * P],
                                x_t[:, ci, :],
                                start=(ci == 0), stop=(ci == CI_CH - 1),
                            )
                    q_sb = qp.tile([P, Q_CH, F], F32R, tag="q")
                    for qi in range(Q_CH):
                        nc.scalar.activation(
                            q_sb[:, qi, :], q_ps[:, qi, :],
                            mybir.ActivationFunctionType.Identity,
                            bias=bq[:, qi:qi + 1],
                        )

                    # simT[k, n] = keym^T-contract-q @ Q
                    sim = sdps.tile([K, F], F32, tag="sd")
                    den = denps.tile([1, F], F32, tag="den")
                    for qi in range(Q_CH):
                        nc.tensor.matmul(
                            sim, keym[:, qi, :], q_sb[:, qi, :],
                            start=(qi == 0), stop=(qi == Q_CH - 1),
                        )
                    e = ep.tile([K, F], F32R, tag="e")
                    nc.scalar.activation(
                        e, sim, mybir.ActivationFunctionType.Exp, scale=SCALE
                    )
                    nc.tensor.matmul(den, ones_col, e, start=True, stop=True)
                    r_sb = rp.tile([1, F], F32R, tag="r")
                    with nc.allow_low_precision(reason="f32r is 4-byte fp32"):
                        nc.vector.reciprocal(r_sb, den)
                    rb_ps = rbps.tile([K, F], F32, tag="rb")
                    nc.tensor.matmul(rb_ps, ones_row[:, :K], r_sb, start=True, stop=True)
                    en = enp.tile([K, F], F32R, tag="en")
                    nc.vector.tensor_tensor(en, rb_ps, e, mybir.AluOpType.mult)

                    # ctxT[v, n] = value^T-contract-k @ expPn -> [256, F]
                    ctx_ps = ctxps.tile([P, V_CH, F], F32, tag="ctxps")
                    for vi in range(V_CH):
                        nc.tensor.matmul(
                            ctx_ps[:, vi, :],
                            value[:, vi * P:(vi + 1) * P],
                            en,
                            start=True, stop=True,
                        )
                    ctx_sb = ctxp.tile([P, V_CH, F], F32R, tag="ctx")
                    nc.scalar.copy(ctx_sb, ctx_ps)

                    # out = Wo @ ctx -> [512, F]
                    out_ps = outps.tile([P, O_CH, F], F32, tag="outps")
                    for oi in range(O_CH):
                        for vi in range(V_CH):
                            nc.tensor.matmul(
                                out_ps[:, oi, :],
                                wo[:, vi, oi * P:(oi + 1) * P],
                                ctx_sb[:, vi, :],
                                start=(vi == 0), stop=(vi == V_CH - 1),
                            )
                    out_sb = outp.tile([P, O_CH, F], F32, tag="out")
                    nc.vector.tensor_tensor(
                        out_sb, out_ps,
                        bo[:, :, None].to_broadcast([P, O_CH, F]),
                        mybir.AluOpType.add,
                    )
                    nc.sync.dma_start(out_r[:, :, n0:n0 + F], out_sb)

    nc.compile()
    return nc


def _get_nc():
    global _CACHED
    if _CACHED is None:
        _CACHED = _build()
    return _CACHED


def kernel(x, proxy, Wq, bq, Wk, bk, Wv, bv, Wo, bo, **run_kwargs):
    nc = _get_nc()

    shared = {
        "wqT": np.ascontiguousarray(Wq.T).astype(np.float16),
        "wkT": np.ascontiguousarray(Wk.T).astype(np.float32),
        "wvT": np.ascontiguousarray(Wv.T).astype(np.float32),
        "woT": np.ascontiguousarray(Wo.T).astype(np.float32),
        "bq": np.ascontiguousarray(bq).astype(np.float32),
        "bk": np.ascontiguousarray(bk).astype(np.float32).reshape(1, CK),
        "bv": np.ascontiguousarray(bv).astype(np.float32).reshape(1, CV),
        "bo": np.ascontiguousarray(bo).astype(np.float32),
        "ones_row": np.ones((1, 256), np.float32),
        "ones_col": np.ones((K, 1), np.float32),
    }
    in_maps = []
    for b in range(B):
        m = dict(shared)
        m["x"] = np.ascontiguousarray(x[b]).reshape(C, N).astype(np.float16)
        m["pf"] = np.ascontiguousarray(proxy[b, :, :, 0]).astype(np.float32)
        in_maps.append(m)

    res = bass_utils.run_bass_kernel_spmd(
        nc, in_maps, core_ids=list(range(B)), **run_kwargs
    )
    out = np.stack([res.results[b]["out"] for b in range(B)], axis=0)
    if run_kwargs:
        kernel.last_results = res
    return out.reshape(B, C, H, W)
